# revision 9
# baseline (speedup 1.0000x reference)
"""Multi-head attention + residual + LayerNorm on 8 trn2 NeuronCores.

Problem: B=4, S=1024, D=1024, H=16 (DH=64).
  q = query @ Wq.T + bq ; k, v likewise
  reshape to [B*H, S, DH] via RAW row-major reshape (preserved source bug):
    head h of batch b = rows 64h:64h+64 of q[b], reshaped to [1024, 64];
    s' = r*16 + c maps to (row 64h+r, cols 64c:64c+64).
  attn = softmax(q_h @ k_h.T * D**-0.5)  -> output #2 [B*H, S, S]
  context_h -> rows 64h:64h+64 of context[b]
  out = LayerNorm(query + context) * gamma + beta -> output #1

Sharding: head h only touches rows 64h:64h+64 of q/k/v[b], so core
(b, half) = (core//2, core%2) owns heads 8*half..+8 completely =
rows 512*half..+512 of batch b. No duplicated work, no collectives;
the host gather is pure concatenation.

All matmuls run in float32r (1 cycle/row at N>=512, ~1.5e-4 relative
error) and must be entirely partition-base-0 (walrus rejects f32r with
any nonzero tile_position). On-chip per-head operands use the tau order
tau = c*64 + r for s' = r*16 + c; they are built with SBUF->SBUF DMAs
(the only lane-crossing copy). The attn DRAM write fixes the s' order
with strided APs on the exp eviction (free axis) and the DMA
(partition axis).
"""
import numpy as np

import concourse.bass as bass
import concourse.mybir as mybir
import concourse.tile as tile
import bass_rust
from concourse.bass_utils import run_bass_kernel_spmd
from concourse.masks import make_identity

dt = mybir.dt
AF = mybir.ActivationFunctionType
ALU = mybir.AluOpType

B, S, D, H = 4, 1024, 1024, 16
DH = D // H          # 64
P = 128
SQ = 512             # rows per core
NDB = D // P         # 8
NH = 8               # local heads per core
SCALE = float(D) ** -0.5
EPS = 1e-5
f32 = dt.float32
f32r = dt.float32r


def _split_excess_waits(nc, max_waits=1):
    """walrus CoreV3 rejects CTRL ops carrying >2 sem waits; hoist excess
    waits onto NoOps inserted before the instruction on the same engine."""
    seq = 0
    for f in nc.m.functions:
        for bb in f.blocks:
            insts = list(bb.instructions)
            new = []
            changed = False
            for inst in insts:
                si = inst.sync_info
                waits = list(si.on_wait) if si and si.on_wait else []
                if len(waits) > max_waits:
                    changed = True
                    head, rest = waits[:-max_waits], waits[-max_waits:]
                    for i in range(0, len(head), max_waits):
                        seq += 1
                        nop = mybir.InstNoOp(name=f"I-ws-{seq}", ins=[], outs=[])
                        nop.engine = inst.engine
                        nop.sync_info = bass_rust.SyncInfo(
                            on_wait=head[i : i + max_waits], on_update=[]
                        )
                        new.append(nop)
                    si.on_wait = rest
                new.append(inst)
            if changed:
                bb.instructions = new
    return nc


def build_nc():
    nc = bass.Bass()
    q_in = nc.declare_dram_parameter("q_in", [SQ, D], f32, isOutput=False)
    k_in = nc.declare_dram_parameter("k_in", [SQ, D], f32, isOutput=False)
    v_in = nc.declare_dram_parameter("v_in", [SQ, D], f32, isOutput=False)
    wq = nc.declare_dram_parameter("wq", [D, D], f32, isOutput=False)
    wk = nc.declare_dram_parameter("wk", [D, D], f32, isOutput=False)
    wv = nc.declare_dram_parameter("wv", [D, D], f32, isOutput=False)
    bq = nc.declare_dram_parameter("bq", [1, D], f32, isOutput=False)
    bk = nc.declare_dram_parameter("bk", [1, D], f32, isOutput=False)
    bv = nc.declare_dram_parameter("bv", [1, D], f32, isOutput=False)
    gamma = nc.declare_dram_parameter("gamma", [1, D], f32, isOutput=False)
    beta = nc.declare_dram_parameter("beta", [1, D], f32, isOutput=False)
    attn_out = nc.declare_dram_parameter("attn_out", [NH, S, S], f32, isOutput=True)
    y_out = nc.declare_dram_parameter("y_out", [SQ, D], f32, isOutput=True)

    with tile.TileContext(nc) as tc:
        with (
            tc.tile_pool(name="const", bufs=1) as const,
            tc.tile_pool(name="proj", bufs=1) as proj,
            tc.tile_pool(name="pbig", bufs=2, space="PSUM") as pbig,
            tc.tile_pool(name="pctx", bufs=1, space="PSUM") as pctx,
            tc.tile_pool(name="ptiny", bufs=2, space="PSUM") as ptiny,
        ):
            ident = const.tile([P, P], f32)
            make_identity(nc, ident[:])
            eps_t = const.tile([P, 1], f32)
            nc.vector.memset(eps_t[:], EPS)

            # bias per j-block: bqc[p, jb] = bias[jb*128 + p]
            bqc = const.tile([P, NDB], f32)
            bkc = const.tile([P, NDB], f32)
            nc.sync.dma_start(bqc[:], bq[0].rearrange("(a b) -> b a", b=P))
            nc.sync.dma_start(bkc[:], bk[0].rearrange("(a b) -> b a", b=P))
            bv_b = const.tile([P, D], f32)
            gm_b = const.tile([P, D], f32)
            bt_b = const.tile([P, D], f32)
            nc.sync.dma_start(bv_b[:], bv[0:1, :].broadcast_to([P, D]))
            nc.sync.dma_start(gm_b[:], gamma[0:1, :].broadcast_to([P, D]))
            nc.sync.dma_start(bt_b[:], beta[0:1, :].broadcast_to([P, D]))

            # persistent projected tensors (f32r):
            # qT/kT: [p = j%128, jb*512 + s];  vS: [p = s%128, sb*1024 + j]
            qT = proj.tile([P, NDB * SQ], f32r)
            kT = proj.tile([P, NDB * SQ], f32r)
            vS = proj.tile([P, 4 * D], f32r)

            def transpose_in(x_dram, n_rows, xT_tile, blk_cols):
                """x [n_rows, D] -> xT d-major: col db*blk_cols + row."""
                with tc.tile_pool(name="strip", bufs=2) as strips:
                    for rb in range(n_rows // P):
                        st = strips.tile([P, D], f32, tag="strip")
                        nc.sync.dma_start(st[:], x_dram[rb * P : (rb + 1) * P, :])
                        for db in range(NDB):
                            pt = ptiny.tile([P, P], f32, tag="tp")
                            nc.tensor.transpose(
                                pt[:], st[:, db * P : (db + 1) * P], ident[:]
                            )
                            nc.scalar.copy(
                                xT_tile[:, db * blk_cols + rb * P : db * blk_cols + (rb + 1) * P],
                                pt[:],
                            )

            def project(w_dram, x_dram, out, bias_col=None, bias_row=None):
                """out[j-major] or vS[s-major] = x @ W.T + b."""
                with tc.tile_pool(name="wt", bufs=1) as wtp:
                    WT = wtp.tile([P, NDB * D], f32r, tag="wt")
                    transpose_in(w_dram, D, WT, D)
                    xT = wtp.tile([P, NDB * SQ], f32r, tag="xt")
                    transpose_in(x_dram, SQ, xT, SQ)
                    if bias_col is not None:      # q/k: out = [j%128, jb*512+s]
                        for jb in range(NDB):
                            pt = pbig.tile([P, SQ], f32, tag="big")
                            for db in range(NDB):
                                nc.tensor.matmul(
                                    pt[:, 0:SQ],
                                    WT[:, db * D + jb * P : db * D + (jb + 1) * P],
                                    xT[:, db * SQ : (db + 1) * SQ],
                                    start=(db == 0),
                                    stop=(db == NDB - 1),
                                )
                            nc.vector.tensor_scalar_add(
                                out[:, jb * SQ : (jb + 1) * SQ], pt[:, 0:SQ],
                                bias_col[:, jb : jb + 1],
                            )
                    else:                          # v: out = [s%128, sb*1024+j]
                        for sb in range(4):
                            for jc in range(2):
                                pt = pbig.tile([P, 512], f32, tag="big")
                                for db in range(NDB):
                                    nc.tensor.matmul(
                                        pt[:, 0:512],
                                        xT[:, db * SQ + sb * P : db * SQ + (sb + 1) * P],
                                        WT[:, db * D + jc * 512 : db * D + (jc + 1) * 512],
                                        start=(db == 0),
                                        stop=(db == NDB - 1),
                                    )
                                nc.vector.tensor_tensor(
                                    out=out[:, sb * D + jc * 512 : sb * D + (jc + 1) * 512],
                                    in0=pt[:, 0:512],
                                    in1=bias_row[:, jc * 512 : (jc + 1) * 512],
                                    op=ALU.add,
                                )

            project(wq, q_in, qT, bias_col=bqc)
            project(wk, k_in, kT, bias_col=bkc)
            project(wv, v_in, vS, bias_row=bv_b)

            # ---- attention, per local head ----
            with (
                tc.tile_pool(name="rsig", bufs=1) as rsigp,
                tc.tile_pool(name="hcp", bufs=2) as hcp,
                tc.tile_pool(name="soft", bufs=3) as soft,
                tc.tile_pool(name="ebp", bufs=3) as ebp,
                tc.tile_pool(name="ctp", bufs=2) as ctp,
                tc.tile_pool(name="lnp", bufs=2) as lnp,
                tc.tile_pool(name="smol", bufs=2) as smol,
            ):
                rsig = rsigp.tile([P, NH * 8], f32)  # col h*8 + pb

                for h in range(NH):
                    hp = h % 2
                    # per-head tau-order operands via SBUF->SBUF DMA
                    # qhT[d', c*64+r] = qT[(c%2)*64+d', (c//2)*512 + 64h + r]
                    qhT = hcp.tile([64, S], f32r, tag="qhT")
                    khT = hcp.tile([64, S], f32r, tag="khT")
                    for dst, srct in ((qhT, qT), (khT, kT)):
                        for par in range(2):
                            ov = dst[:].rearrange(
                                "p (c q r) -> p q c r", c=8, q=2, r=64
                            )[:, par]
                            iv = srct[64 * par : 64 * par + 64, :].rearrange(
                                "p (c s) -> p c s", c=8
                            )[:, :, 64 * h : 64 * h + 64]
                            nc.sync.dma_start(ov, iv)
                    # vht[c2lo*64+r2, pb2*64+d'] =
                    #   vS[hp*64+r2, (h//2)*1024 + (2*pb2+c2lo)*64 + d']
                    vht = hcp.tile([P, 512], f32r, tag="vht")
                    for c2lo in range(2):
                        iv = vS[64 * hp : 64 * hp + 64, :].rearrange(
                            "p (sb c q dd) -> p sb c q dd", sb=4, c=8, q=2, dd=64
                        )[:, h // 2, :, c2lo]
                        nc.sync.dma_start(vht[64 * c2lo : 64 * c2lo + 64, :], iv)

                    # (a) scores [tau-s1' part, sk' free] + softmax + attn out
                    for pb in range(8):
                        pa = pbig.tile([P, S], f32, tag="big")
                        for ck in range(2):
                            nc.tensor.matmul(
                                pa[:, ck * 512 : (ck + 1) * 512],
                                qhT[:, pb * 128 : (pb + 1) * 128],
                                khT[:, ck * 512 : (ck + 1) * 512],
                                start=True, stop=True,
                            )
                        ea = soft.tile([P, S], f32, tag="ea")
                        sumexp = smol.tile([P, 1], f32, tag="sum")
                        # free reorder: psum col c2*64+r2 -> sk' = r2*16+c2
                        ea_v = ea[:].rearrange("p (r c) -> p c r", r=64, c=16)
                        nc.scalar.activation(
                            ea_v, pa[:], AF.Exp, scale=SCALE, accum_out=sumexp[:]
                        )
                        nc.vector.reciprocal(
                            rsig[:, h * 8 + pb : h * 8 + pb + 1], sumexp[:]
                        )
                        at = soft.tile([P, S], f32, tag="attn")
                        nc.vector.tensor_scalar_mul(
                            at[:], ea[:], rsig[:, h * 8 + pb : h * 8 + pb + 1]
                        )
                        # partition p = c1lo*64+r1 -> s1' = r1*16 + 2*pb + c1lo
                        # (one DMA per c1lo: DMA partition axis must be 1-dim)
                        for i in range(2):
                            dv = attn_out[h].rearrange(
                                "(r c) k -> c r k", r=64, c=16
                            )[2 * pb + i]
                            nc.sync.dma_start(dv, at[64 * i : 64 * i + 64, :])

                    # r64[r1, c1] = rsig[(c1%2)*64+r1, h*8 + c1//2]
                    t1 = ptiny.tile([8, P], f32, tag="tp")
                    nc.tensor.transpose(t1[:], rsig[:, h * 8 : h * 8 + 8], ident[:])
                    t1s = smol.tile([8, P], f32, tag="t1s")
                    nc.vector.tensor_copy(t1s[:], t1[:])
                    r64 = smol.tile([64, 16], f32, tag="r64")
                    r64v = r64[:].rearrange("p (b i) -> p i b", b=8, i=2)
                    for chi in range(2):
                        t2 = ptiny.tile([64, 8], f32, tag="tp")
                        nc.tensor.transpose(
                            t2[:], t1s[:, 64 * chi : 64 * chi + 64], ident[0:8, 0:8]
                        )
                        nc.vector.tensor_copy(r64v[:, chi], t2[:])

                    # (b) exp(scores^T) + (c) context^T accumulation
                    pcs = pctx.tile([64, S], f32, tag="pc")
                    for pb2 in range(8):
                        pbm = pbig.tile([P, S], f32, tag="big")
                        for ck in range(2):
                            nc.tensor.matmul(
                                pbm[:, ck * 512 : (ck + 1) * 512],
                                khT[:, pb2 * 128 : (pb2 + 1) * 128],
                                qhT[:, ck * 512 : (ck + 1) * 512],
                                start=True, stop=True,
                            )
                        eb = ebp.tile([P, S], f32r, tag="eb")
                        nc.scalar.activation(eb[:], pbm[:], AF.Exp, scale=SCALE)
                        for ck in range(2):
                            nc.tensor.matmul(
                                pcs[:, ck * 512 : (ck + 1) * 512],
                                vht[:, pb2 * 64 : (pb2 + 1) * 64],
                                eb[:, ck * 512 : (ck + 1) * 512],
                                start=(pb2 == 0),
                                stop=(pb2 == 7),
                            )
                    cT = ctp.tile([64, S], f32, tag="cT")
                    nc.scalar.copy(cT[:], pcs[:])

                    # (d) ctx^T -> ctx rows:  ctxh[r1, 64c1+d'] = cT[d', c1*64+r1]
                    ctxh = ctp.tile([64, D], f32, tag="ctxh")
                    for c1 in range(16):
                        pt2 = ptiny.tile([64, 64], f32, tag="tp")
                        nc.tensor.transpose(
                            pt2[:], cT[:, c1 * 64 : (c1 + 1) * 64], ident[0:64, 0:64]
                        )
                        nc.vector.tensor_copy(ctxh[:, 64 * c1 : 64 * (c1 + 1)], pt2[:])

                    # scale by 1/sumexp, add residual, layernorm
                    resid = lnp.tile([64, D], f32, tag="resid")
                    nc.sync.dma_start(resid[:], q_in[64 * h : 64 * h + 64, :])
                    x = lnp.tile([64, D], f32, tag="x")
                    r64b = r64[:, :, None].broadcast_to([64, 16, 64])
                    nc.vector.tensor_tensor(
                        out=x[:].rearrange("p (a b) -> p a b", a=16),
                        in0=ctxh[:].rearrange("p (a b) -> p a b", a=16),
                        in1=r64b, op=ALU.mult,
                    )
                    nc.vector.tensor_tensor(out=x[:], in0=x[:], in1=resid[:], op=ALU.add)
                    musum = smol.tile([64, 1], f32, tag="musum")
                    nc.vector.reduce_sum(musum[:], x[:], axis=mybir.AxisListType.X)
                    mu = smol.tile([64, 1], f32, tag="mu")
                    nc.scalar.mul(mu[:], musum[:], 1.0 / D)
                    xc = lnp.tile([64, D], f32, tag="xc")
                    nc.vector.tensor_scalar_sub(xc[:], x[:], mu[:])
                    sq2 = lnp.tile([64, D], f32, tag="y")
                    varsum = smol.tile([64, 1], f32, tag="varsum")
                    nc.scalar.activation(sq2[:], xc[:], AF.Square, accum_out=varsum[:])
                    sd = smol.tile([64, 1], f32, tag="sd")
                    nc.scalar.activation(
                        sd[:], varsum[:], AF.Sqrt, scale=1.0 / D, bias=eps_t[0:64, :]
                    )
                    rstd = smol.tile([64, 1], f32, tag="rstd")
                    nc.vector.reciprocal(rstd[:], sd[:])
                    y1 = lnp.tile([64, D], f32, tag="y")
                    nc.vector.tensor_scalar(
                        out=y1[:], in0=xc[:], scalar1=rstd[:], scalar2=None,
                        op0=ALU.mult,
                    )
                    nc.vector.tensor_tensor(out=y1[:], in0=y1[:], in1=gm_b[0:64, :], op=ALU.mult)
                    nc.vector.tensor_tensor(out=y1[:], in0=y1[:], in1=bt_b[0:64, :], op=ALU.add)
                    nc.sync.dma_start(y_out[64 * h : 64 * h + 64, :], y1[:])
    _split_excess_waits(nc)
    return nc


_NC = None


def _in_maps(key, value, query, Wq, bq, Wk, bk, Wv, bv, gamma, beta):
    def row(v):
        return np.ascontiguousarray(np.asarray(v, dtype=np.float32).reshape(1, D))

    Wq = np.ascontiguousarray(np.asarray(Wq, dtype=np.float32))
    Wk = np.ascontiguousarray(np.asarray(Wk, dtype=np.float32))
    Wv = np.ascontiguousarray(np.asarray(Wv, dtype=np.float32))
    bqv, bkv, bvv, gm, bt = row(bq), row(bk), row(bv), row(gamma), row(beta)
    maps = []
    for c in range(8):
        b, half = c // 2, c % 2
        rows = slice(half * SQ, half * SQ + SQ)
        maps.append({
            "q_in": np.ascontiguousarray(np.asarray(query, np.float32)[b, rows]),
            "k_in": np.ascontiguousarray(np.asarray(key, np.float32)[b, rows]),
            "v_in": np.ascontiguousarray(np.asarray(value, np.float32)[b, rows]),
            "wq": Wq, "wk": Wk, "wv": Wv,
            "bq": bqv, "bk": bkv, "bv": bvv, "gamma": gm, "beta": bt,
        })
    return maps


def kernel(key, value, query, Wq, bq, Wk, bk, Wv, bv, gamma, beta):
    global _NC
    if _NC is None:
        _NC = build_nc()
    maps = _in_maps(key, value, query, Wq, bq, Wk, bk, Wv, bv, gamma, beta)
    res = run_bass_kernel_spmd(_NC, maps, list(range(8)))
    output = np.empty((B, S, D), dtype=np.float32)
    attn = np.empty((B * H, S, S), dtype=np.float32)
    for c in range(8):
        b, half = c // 2, c % 2
        r = res.results[c]
        attn[b * H + half * NH : b * H + half * NH + NH] = r["attn_out"]
        output[b, half * SQ : half * SQ + SQ, :] = r["y_out"]
    return output, attn
